# revision 1
# baseline (speedup 1.0000x reference)
"""GNN message-passing kernel for Trainium2 (8 NeuronCores, SPMD).

Strategy (edge-parallel by destination):
  * Host sorts edges by destination node, assigns 128-node blocks to
    (core, window-slot) pairs so per-slot edge counts are balanced across
    cores (one shared compile-time tile schedule for all 8 cores).
  * Host pre-gathers x[row] / edge_attr per edge (transposed, bf16), and
    precomputes per-edge scale wrc = wts / max(count[col], 1) so the device
    scatter directly accumulates mean-normalized messages.
  * Device, per 128-edge tile: h = eaxT.T @ mw1_aug (PSUM, 4 tiles share a
    [128, 512] bank), G = relu(h) (one ACT per 4 tiles, bf16), S_w =
    (iota == colof) * wrc (weighted one-hot, DVE), then scatter-accumulate
    T_wT[hid, node] += G.T @ S_w in PSUM per 128-node window.
  * Per window: recvT = mw2.T @ T_wT, update MLP on [x; recv; u; s; 1]
    with biases folded into augmented weight rows, out written transposed.
  * Host inverts the node permutation and returns [N, 64] float32.
"""
import numpy as np
import ml_dtypes

import concourse.bacc as bacc
import concourse.tile as tile
from concourse import mybir
from concourse.bass_utils import run_bass_kernel_spmd

BF = mybir.dt.bfloat16
F32 = mybir.dt.float32
bf16 = ml_dtypes.bfloat16

P = 128
NCORES = 8
HID = 128
NODE_D = 64
EDGE_D = 32
GLOB_D = 32
FEAT = NODE_D + EDGE_D + 1   # 97: x | edge_attr | ones
UPB = GLOB_D + 2             # 34: u | s | ones
GROUP = 16                   # 128-edge tiles per eax/cw DMA group
AGRP = 4                     # tiles sharing one PSUM bank + one ACT relu
WGRP = 4                     # windows per xua/xub/out DMA group

# const blob column layout (bf16, [128, 640])
_B_MW1 = 0      # [0:97, 0:128]    mw1_aug
_B_MW2 = 128    # [0:128, 128:192] mw2
_B_UW1A = 192   # [0:128, 192:320] uw1 rows 0:128 (x-part rows 0:64, recv 64:128)
_B_UW1B = 320   # [0:34, 320:448]  uw1 rows 128:160 (u part) | v | ub1
_B_UW2 = 448    # [0:128, 448:512] uw2
_B_IOTA = 512   # [0:128, 512:640] iota row 0..127
_B_UW1R = 640   # [0:64, 640:768]  uw1 rows 64:128 (recv part, base partition 0)
BLOB_W = 768

# tunables for sim sweeps (defaults used by the grading path)
CFG = {
    "geax": 3, "gcw": 3, "gx": 2, "ework": 2, "swork": 6, "nwork": 3,
    "gout": 2, "ph": 2, "pt": 2,
    "hh_engine": "dve",   # dve | act
    "h2r_engine": "dve",  # dve | act
    }

_program_cache: dict = {}
_last_results = None  # BassKernelResults of the most recent run (for profiling)


def _build_program(t_sched):
    """Build + finalize the SPMD Bass program for a tile schedule."""
    nt = sum(t_sched)
    e_pad = nt * P
    nslots = len(t_sched)
    nsh = nslots * P
    assert nt % GROUP == 0

    nc = bacc.Bacc()
    eax_d = nc.dram_tensor("eax", [FEAT, e_pad], BF, kind="ExternalInput")
    cw_d = nc.dram_tensor("cw", [nt, P, 2], F32, kind="ExternalInput")
    blob_d = nc.dram_tensor("blob", [P, BLOB_W], BF, kind="ExternalInput")
    ub2_d = nc.dram_tensor("ub2", [64, 1], F32, kind="ExternalInput")
    xua_d = nc.dram_tensor("xua", [NODE_D, nsh], BF, kind="ExternalInput")
    xub_d = nc.dram_tensor("xub", [UPB, nsh], BF, kind="ExternalInput")
    out_d = nc.dram_tensor("out", [64, nsh], F32, kind="ExternalOutput")

    with tile.TileContext(nc) as tc:
        with (
            tc.tile_pool(name="consts", bufs=1) as consts,
            tc.tile_pool(name="geax", bufs=CFG["geax"]) as geax,
            tc.tile_pool(name="gcw", bufs=CFG["gcw"]) as gcw,
            tc.tile_pool(name="gx", bufs=CFG["gx"]) as gx,
            tc.tile_pool(name="ework", bufs=CFG["ework"]) as ework,
            tc.tile_pool(name="swork", bufs=CFG["swork"]) as swork,
            tc.tile_pool(name="nwork", bufs=CFG["nwork"]) as nwork,
            tc.tile_pool(name="gout", bufs=CFG["gout"]) as gout,
            tc.tile_pool(name="ph", bufs=CFG["ph"], space="PSUM") as ph,
            tc.tile_pool(name="pt", bufs=CFG["pt"], space="PSUM") as pt,
            tc.tile_pool(name="pr", bufs=1, space="PSUM") as pr,
            tc.tile_pool(name="p2", bufs=1, space="PSUM") as p2,
            tc.tile_pool(name="po", bufs=1, space="PSUM") as po,
        ):
            blob_t = consts.tile([P, BLOB_W], BF)
            nc.sync.dma_start(blob_t[:], blob_d[:])
            mw1_t = blob_t[0:FEAT, _B_MW1:_B_MW1 + HID]
            mw2_t = blob_t[0:HID, _B_MW2:_B_MW2 + 64]
            uw1ax_t = blob_t[0:NODE_D, _B_UW1A:_B_UW1A + HID]
            uw1ar_t = blob_t[0:NODE_D, _B_UW1R:_B_UW1R + HID]
            uw1b_t = blob_t[0:UPB, _B_UW1B:_B_UW1B + HID]
            uw2_t = blob_t[0:HID, _B_UW2:_B_UW2 + 64]
            iota_t = blob_t[:, _B_IOTA:_B_IOTA + P]
            ub2_t = consts.tile([64, 1], F32)
            nc.sync.dma_start(ub2_t[:], ub2_d[:])

            eax_g = cw_g = None
            h4_ps = g4_t = None
            xua_g = xub_g = None
            o_g = None
            t = 0
            for j in range(nslots):
                tj = t_sched[j]
                jg, jr = divmod(j, WGRP)
                if jr == 0:
                    nw = min(WGRP, nslots - jg * WGRP)
                    xua_g = gx.tile([NODE_D, WGRP * P], BF, tag="xua")
                    nc.gpsimd.dma_start(
                        xua_g[:, 0:nw * P],
                        xua_d[:, jg * WGRP * P:(jg * WGRP + nw) * P],
                    )
                    xub_g = gx.tile([UPB, WGRP * P], BF, tag="xub")
                    nc.gpsimd.dma_start(
                        xub_g[:, 0:nw * P],
                        xub_d[:, jg * WGRP * P:(jg * WGRP + nw) * P],
                    )
                    o_g = gout.tile([64, WGRP * P], F32, tag="o")
                t_ps = pt.tile([HID, P], F32, space="PSUM")
                kdone = 0
                while kdone < tj:
                    nk = min(AGRP, tj - kdone)
                    h4_ps = ph.tile([P, AGRP * HID], F32, space="PSUM")
                    chunk = []
                    for i in range(nk):
                        g, r = divmod(t, GROUP)
                        if r == 0:
                            n = min(GROUP, nt - g * GROUP)
                            eax_g = geax.tile([FEAT, GROUP * P], BF, tag="eax")
                            nc.sync.dma_start(
                                eax_g[:, 0:n * P],
                                eax_d[:, g * GROUP * P:(g * GROUP + n) * P],
                            )
                            cw_g = gcw.tile([P, GROUP, 2], F32, tag="cw")
                            nc.sync.dma_start(
                                cw_g[:, 0:n, :],
                                cw_d[g * GROUP:g * GROUP + n, :, :].rearrange(
                                    "t p c -> p t c"
                                ),
                            )
                        nc.tensor.matmul(
                            h4_ps[:, i * HID:(i + 1) * HID],
                            lhsT=eax_g[:, r * P:(r + 1) * P], rhs=mw1_t,
                            start=True, stop=True,
                        )
                        s_t = swork.tile([P, P], BF, tag="S")
                        nc.vector.tensor_scalar(
                            out=s_t[:], in0=iota_t, scalar1=cw_g[:, r, 0:1],
                            scalar2=cw_g[:, r, 1:2],
                            op0=mybir.AluOpType.is_equal, op1=mybir.AluOpType.mult,
                        )
                        chunk.append((i, s_t))
                        t += 1
                    g4_t = ework.tile([P, AGRP * HID], BF, tag="G")
                    nc.scalar.activation(
                        g4_t[:, 0:nk * HID], h4_ps[:, 0:nk * HID],
                        mybir.ActivationFunctionType.Relu,
                    )
                    for i, s_t in chunk:
                        nc.tensor.matmul(
                            t_ps[:], lhsT=g4_t[:, i * HID:(i + 1) * HID],
                            rhs=s_t[:],
                            start=(kdone + i == 0), stop=(kdone + i == tj - 1),
                        )
                    kdone += nk
                # per-window update MLP
                hh = nwork.tile([HID, P], BF, tag="Hh")
                if CFG["hh_engine"] == "act":
                    nc.scalar.copy(hh[:], t_ps[:])
                else:
                    nc.vector.tensor_copy(hh[:], t_ps[:])
                r_ps = pr.tile([64, P], F32, space="PSUM")
                nc.tensor.matmul(r_ps[:], lhsT=mw2_t, rhs=hh[:], start=True,
                                 stop=True)
                recv_sb = nwork.tile([64, P], BF, tag="recv")
                nc.vector.tensor_copy(recv_sb[:], r_ps[:])
                h2_ps = p2.tile([HID, P], F32, space="PSUM")
                nc.tensor.matmul(h2_ps[:], lhsT=uw1ax_t,
                                 rhs=xua_g[:, jr * P:(jr + 1) * P],
                                 start=True, stop=False)
                nc.tensor.matmul(h2_ps[:], lhsT=uw1ar_t, rhs=recv_sb[:],
                                 start=False, stop=False)
                nc.tensor.matmul(h2_ps[:], lhsT=uw1b_t,
                                 rhs=xub_g[:, jr * P:(jr + 1) * P],
                                 start=False, stop=True)
                h2r = nwork.tile([HID, P], BF, tag="h2r")
                if CFG["h2r_engine"] == "act":
                    nc.scalar.activation(
                        h2r[:], h2_ps[:], mybir.ActivationFunctionType.Relu
                    )
                else:
                    nc.vector.tensor_scalar_max(h2r[:], h2_ps[:], 0.0)
                o_ps = po.tile([64, P], F32, space="PSUM")
                nc.tensor.matmul(o_ps[:], lhsT=uw2_t, rhs=h2r[:], start=True,
                                 stop=True)
                nc.vector.tensor_scalar(
                    out=o_g[:, jr * P:(jr + 1) * P], in0=o_ps[:],
                    scalar1=ub2_t[:, 0:1], scalar2=None,
                    op0=mybir.AluOpType.add,
                )
                if jr == nw - 1:
                    nc.gpsimd.dma_start(
                        out_d[:, jg * WGRP * P:(jg * WGRP + nw) * P],
                        o_g[:, 0:nw * P],
                    )
    nc.finalize()
    return nc


def _schedule(col, n_nodes):
    """Assign 128-node blocks to (core, slot) and derive the shared
    per-slot tile schedule."""
    nblk = -(-n_nodes // P)
    nslots = -(-nblk // NCORES)
    nblk_pad = nslots * NCORES
    nsh = nslots * P

    blk = (col >> 7).astype(np.int64)
    order = np.argsort(blk, kind="stable")
    bc = np.bincount(blk, minlength=nblk_pad)
    bstart = np.zeros(nblk_pad + 1, np.int64)
    np.cumsum(bc, out=bstart[1:])

    sorted_blocks = np.argsort(-bc, kind="stable")
    blk_assign = sorted_blocks.reshape(nslots, NCORES)   # [slot, core]
    grp_max = bc[blk_assign].max(axis=1)
    t_sched = np.maximum(1, -(-grp_max // P)).astype(np.int64)
    pad = (-int(t_sched.sum())) % GROUP
    t_sched[-1] += pad
    t_sched = [int(v) for v in t_sched]
    return t_sched, blk_assign, order, bc, bstart, nslots, nsh


def kernel(x, edge_index, edge_attr, u, node_batch, wts,
           mw1, mb1, mw2, mb2, uw1, ub1, uw2, ub2):
    x = np.asarray(x, np.float32)
    edge_index = np.asarray(edge_index)
    edge_attr = np.asarray(edge_attr, np.float32)
    u = np.asarray(u, np.float32)
    node_batch = np.asarray(node_batch).astype(np.int64)
    wts = np.asarray(wts, np.float32).reshape(-1)
    mw1 = np.asarray(mw1, np.float32)
    mb1 = np.asarray(mb1, np.float32)
    mw2 = np.asarray(mw2, np.float32)
    mb2 = np.asarray(mb2, np.float32)
    uw1 = np.asarray(uw1, np.float32)
    ub1 = np.asarray(ub1, np.float32)
    uw2 = np.asarray(uw2, np.float32)
    ub2 = np.asarray(ub2, np.float32)

    n_nodes = x.shape[0]
    row = np.asarray(edge_index[0], np.int64)
    col = np.asarray(edge_index[1], np.int64)

    sched = _schedule(col, n_nodes)
    (t_sched, blk_assign, order, bc, bstart, nslots, nsh) = sched
    nt = sum(t_sched)
    e_pad = nt * P

    # per-node stats (host): count, 1/max(cnt,1), weight-sum
    cnt = np.bincount(col, minlength=n_nodes).astype(np.float32)
    rc = 1.0 / np.maximum(cnt, 1.0)
    wsum = np.bincount(col, weights=wts, minlength=n_nodes).astype(np.float32)
    s_node = wsum * rc

    # per-edge
    colof = (col & 127).astype(np.float32)
    wrc = wts * rc[col]

    key = tuple(t_sched)
    if key not in _program_cache:
        _program_cache[key] = _build_program(t_sched)
    nc = _program_cache[key]

    # const blob (shared by all cores)
    v_row = mb2 @ uw1[NODE_D:2 * NODE_D, :]              # [HID]
    blob = np.zeros((P, BLOB_W), np.float32)
    blob[0:NODE_D + EDGE_D, _B_MW1:_B_MW1 + HID] = mw1
    blob[NODE_D + EDGE_D, _B_MW1:_B_MW1 + HID] = mb1
    blob[0:HID, _B_MW2:_B_MW2 + 64] = mw2
    blob[:, _B_UW1A:_B_UW1A + HID] = uw1[0:2 * NODE_D, :]
    blob[0:NODE_D, _B_UW1R:_B_UW1R + HID] = uw1[NODE_D:2 * NODE_D, :]
    blob[0:GLOB_D, _B_UW1B:_B_UW1B + HID] = uw1[2 * NODE_D:, :]
    blob[GLOB_D, _B_UW1B:_B_UW1B + HID] = v_row
    blob[GLOB_D + 1, _B_UW1B:_B_UW1B + HID] = ub1
    blob[0:HID, _B_UW2:_B_UW2 + 64] = uw2
    blob[:, _B_IOTA:_B_IOTA + P] = np.arange(P, dtype=np.float32)[None, :]
    blob_bf = blob.astype(bf16)
    ub2_a = ub2.reshape(64, 1).astype(np.float32)

    u_per_node = u[node_batch]                           # [N, GLOB_D]

    # slot offsets within a core's edge stream
    slot_off = np.zeros(nslots + 1, np.int64)
    np.cumsum(np.asarray(t_sched) * P, out=slot_off[1:])

    in_maps = []
    node_idx_cores = []
    for c in range(NCORES):
        eidx = np.full(e_pad, -1, np.int64)
        nidx = np.full(nsh, -1, np.int64)
        for j in range(nslots):
            b = int(blk_assign[j, c])
            m = int(bc[b])
            o = slot_off[j]
            eidx[o:o + m] = order[bstart[b]:bstart[b] + m]
            n0 = b * P
            nn = min(P, n_nodes - n0)
            if nn > 0:
                nidx[j * P:j * P + nn] = np.arange(n0, n0 + nn)
        evalid = eidx >= 0
        eidxc = np.where(evalid, eidx, 0)
        # eax: [x[row] | edge_attr | 1] transposed, zeros on pads
        eax = np.empty((e_pad, FEAT), np.float32)
        eax[:, 0:NODE_D] = x[row[eidxc]]
        eax[:, NODE_D:NODE_D + EDGE_D] = edge_attr[eidxc]
        eax[:, FEAT - 1] = 1.0
        eax[~evalid] = 0.0
        cw = np.zeros((e_pad, 2), np.float32)
        cw[evalid, 0] = colof[eidx[evalid]]
        cw[evalid, 1] = wrc[eidx[evalid]]

        nvalid = nidx >= 0
        nidxc = np.where(nvalid, nidx, 0)
        xua = x[nidxc].astype(np.float32)
        xua[~nvalid] = 0.0
        xub = np.zeros((nsh, UPB), np.float32)
        xub[:, 0:GLOB_D] = u_per_node[nidxc]
        xub[:, GLOB_D] = s_node[nidxc]
        xub[:, GLOB_D + 1] = 1.0
        xub[~nvalid] = 0.0

        in_maps.append({
            "eax": np.ascontiguousarray(eax.T).astype(bf16),
            "cw": cw.reshape(nt, P, 2),
            "blob": blob_bf,
            "ub2": ub2_a,
            "xua": np.ascontiguousarray(xua.T).astype(bf16),
            "xub": np.ascontiguousarray(xub.T).astype(bf16),
        })
        node_idx_cores.append((nidx, nvalid))

    res = run_bass_kernel_spmd(nc, in_maps, core_ids=list(range(NCORES)))
    global _last_results
    _last_results = res

    out_full = np.zeros((n_nodes, 64), np.float32)
    for c in range(NCORES):
        nidx, nvalid = node_idx_cores[c]
        oc = res.results[c]["out"]                       # [64, nsh]
        out_full[nidx[nvalid]] = oc.T[nvalid]
    return out_full



# revision 2
# speedup vs baseline: 4.1891x; 4.1891x over previous
"""GNN message-passing kernel for Trainium2 (8 NeuronCores, SPMD).

Strategy (edge-parallel by destination):
  * Host sorts edges by destination node, assigns 128-node blocks to
    (core, window-slot) pairs so per-slot edge counts are balanced across
    cores (one shared compile-time tile schedule for all 8 cores).
  * Host pre-gathers x[row] / edge_attr per edge, folds the per-edge
    scale wrc = wts / max(count[col], 1) directly into the gathered
    features (relu(w*h) == w*relu(h) for w >= 0), and folds mw2 @ uw1r
    into a single weight Wc so the hid-basis scatter feeds the update
    MLP without a per-window mw2 matmul.
  * All heavy DMA goes through the gpsimd software DGE so packets spread
    across all 16 DMA engines (HWDGE queues pin to a single engine).
    cw/xcon are SBUF-resident; eax streams in large groups.
  * Device, per 128-edge tile: h = eaxT.T @ mw1_aug (PSUM), relu on ACT
    (one per 4 tiles), one-hot S built on DVE one instr per 4 tiles in
    an interleaved [node, tile] layout, scatter T_w[hid, node] += G.T @ S.
  * Update MLP per 4-window quad: h2 = uw1aug.T @ xcon + Wc.T @ T,
    relu, out = uw2.T @ h2r + ub2, written bf16.
"""
import numpy as np
import ml_dtypes

import concourse.bacc as bacc
import concourse.tile as tile
from concourse import mybir
from concourse.bass_utils import run_bass_kernel_spmd

BF = mybir.dt.bfloat16
F32 = mybir.dt.float32
bf16 = ml_dtypes.bfloat16

P = 128
NCORES = 8
HID = 128
NODE_D = 64
EDGE_D = 32
GLOB_D = 32
FEAT = 98                    # x*w | ea*w | w | zero-pad  (98 = 2*49)
XCON_R = 98                  # x | u | s | ones
GROUP = 64                   # 128-edge tiles per eax DMA group
AGRP = 4                     # tiles per relu/S-build batch
QUAD = 4                     # windows per update-MLP batch
OGRP = 2                     # quads per output DMA

# const blob column layout (bf16)
_B_MW1 = 0                   # [0:98, 0:128]      mw1_aug (w/ mb1 row)
_B_UW1 = 128                 # [0:98, 128:256]    uw1aug (x|u|v_row|ub1)
_B_WC = 256                  # [0:128, 256:384]   Wc = mw2 @ uw1r
_B_UW2 = 384                 # [0:128, 384:448]   uw2
_B_IR = (448, 576, 832, 1216)  # iotaRep nk=1..4: col c -> c // nk
BLOB_W = 1728

CFG = {
    "geax": 3, "gs": 3, "gg": 3, "gn": 2, "go": 2,
    "ph": 2, "pt": 2, "p2": 2,
}

_program_cache: dict = {}
_last_results = None


def _build_program(t_sched):
    nt = sum(t_sched)
    e_pad = nt * P
    nslots = len(t_sched)
    nsh = nslots * P
    nquads = -(-nslots // QUAD)

    nc = bacc.Bacc()
    eax_d = nc.dram_tensor("eax", [FEAT, e_pad], BF, kind="ExternalInput")
    cwv_d = nc.dram_tensor("cwv", [P, nt], BF, kind="ExternalInput")
    blob_d = nc.dram_tensor("blob", [P, BLOB_W], BF, kind="ExternalInput")
    ub2_d = nc.dram_tensor("ub2", [64, 1], F32, kind="ExternalInput")
    xcon_d = nc.dram_tensor("xcon", [XCON_R, nsh], BF, kind="ExternalInput")
    out_d = nc.dram_tensor("out", [64, nsh], BF, kind="ExternalOutput")

    with tile.TileContext(nc) as tc:
        with (
            tc.tile_pool(name="consts", bufs=1) as consts,
            tc.tile_pool(name="geax", bufs=CFG["geax"]) as geax,
            tc.tile_pool(name="gs", bufs=CFG["gs"]) as gs,
            tc.tile_pool(name="gg", bufs=CFG["gg"]) as gg,
            tc.tile_pool(name="gn", bufs=CFG["gn"]) as gn,
            tc.tile_pool(name="go", bufs=CFG["go"]) as go,
            tc.tile_pool(name="ph", bufs=CFG["ph"], space="PSUM") as ph,
            tc.tile_pool(name="pt", bufs=CFG["pt"], space="PSUM") as pt,
            tc.tile_pool(name="p2", bufs=CFG["p2"], space="PSUM") as p2,
            tc.tile_pool(name="po", bufs=1, space="PSUM") as po,
        ):
            blob_t = consts.tile([P, BLOB_W], BF)
            nc.gpsimd.dma_start(blob_t[:], blob_d[:])
            mw1_t = blob_t[0:FEAT, _B_MW1:_B_MW1 + HID]
            uw1_t = blob_t[0:XCON_R, _B_UW1:_B_UW1 + HID]
            wc_t = blob_t[0:HID, _B_WC:_B_WC + HID]
            uw2_t = blob_t[0:HID, _B_UW2:_B_UW2 + 64]
            ub2_t = consts.tile([64, 1], F32)
            nc.gpsimd.dma_start(ub2_t[:], ub2_d[:])
            cwv_t = consts.tile([P, nt], BF)
            nc.gpsimd.dma_start(cwv_t[:], cwv_d[:])
            xcon_t = consts.tile([XCON_R, nsh], BF)
            nc.gpsimd.dma_start(xcon_t[:], xcon_d[:])

            eax_g = None
            ptq = None
            o_sb = None
            t = 0
            for j in range(nslots):
                tj = t_sched[j]
                q, jr = divmod(j, QUAD)
                if jr == 0:
                    qw = min(QUAD, nslots - q * QUAD)
                    ptq = pt.tile([P, QUAD * P], F32, space="PSUM")
                kdone = 0
                while kdone < tj:
                    nk = min(AGRP, tj - kdone)
                    h4_ps = ph.tile([P, AGRP * HID], F32, space="PSUM")
                    tiles = []
                    for i in range(nk):
                        g, r = divmod(t, GROUP)
                        if r == 0:
                            n = min(GROUP, nt - g * GROUP)
                            eax_g = geax.tile([FEAT, GROUP * P], BF, tag="eax")
                            nc.gpsimd.dma_start(
                                eax_g[:, 0:n * P],
                                eax_d[:, g * GROUP * P:(g * GROUP + n) * P],
                            )
                        nc.tensor.matmul(
                            h4_ps[:, i * HID:(i + 1) * HID],
                            lhsT=eax_g[:, r * P:(r + 1) * P], rhs=mw1_t,
                            start=True, stop=True,
                        )
                        tiles.append((i, eax_g))
                        t += 1
                    # one-hot S for nk tiles, interleaved [node, tile] layout
                    s4 = gs.tile([P, AGRP * P], BF, tag="S")
                    irc = _B_IR[nk - 1]
                    nc.vector.tensor_tensor(
                        s4[:, 0:nk * P].rearrange("p (n t) -> p n t", t=nk),
                        blob_t[:, irc:irc + nk * P].rearrange(
                            "p (n t) -> p n t", t=nk),
                        cwv_t[:, t - nk:t, None]
                        .rearrange("p t o -> p o t")
                        .broadcast_to([P, P, nk]),
                        op=mybir.AluOpType.is_equal,
                    )
                    g4 = gg.tile([P, AGRP * HID], BF, tag="G")
                    nc.scalar.activation(
                        g4[:, 0:nk * HID], h4_ps[:, 0:nk * HID],
                        mybir.ActivationFunctionType.Relu,
                    )
                    s4v = s4[:, 0:nk * P].rearrange("p (n t) -> p n t", t=nk)
                    for i, _ in tiles:
                        nc.tensor.matmul(
                            ptq[:, jr * P:(jr + 1) * P],
                            lhsT=g4[:, i * HID:(i + 1) * HID],
                            rhs=s4v[:, :, i],
                            start=(kdone + i == 0), stop=(kdone + i == tj - 1),
                        )
                    kdone += nk
                # quad boundary: run update MLP on 4 windows at once
                if jr == qw - 1:
                    w = qw * P
                    n0 = q * QUAD * P
                    hh4 = gn.tile([HID, QUAD * P], BF, tag="hh")
                    nc.vector.tensor_copy(hh4[:, 0:w], ptq[:, 0:w])
                    h2_ps = p2.tile([HID, QUAD * P], F32, space="PSUM")
                    nc.tensor.matmul(
                        h2_ps[:, 0:w], lhsT=uw1_t,
                        rhs=xcon_t[:, n0:n0 + w],
                        start=True, stop=False,
                    )
                    nc.tensor.matmul(
                        h2_ps[:, 0:w], lhsT=wc_t, rhs=hh4[:, 0:w],
                        start=False, stop=True,
                    )
                    h2r = gn.tile([HID, QUAD * P], BF, tag="h2r")
                    nc.scalar.activation(
                        h2r[:, 0:w], h2_ps[:, 0:w],
                        mybir.ActivationFunctionType.Relu,
                    )
                    o_ps = po.tile([64, QUAD * P], F32, space="PSUM")
                    nc.tensor.matmul(o_ps[:, 0:w], lhsT=uw2_t,
                                     rhs=h2r[:, 0:w], start=True, stop=True)
                    qo = q % OGRP
                    if qo == 0:
                        o_sb = go.tile([64, OGRP * QUAD * P], BF, tag="o")
                    nc.vector.tensor_scalar(
                        out=o_sb[:, qo * QUAD * P:qo * QUAD * P + w],
                        in0=o_ps[:, 0:w],
                        scalar1=ub2_t[:, 0:1], scalar2=None,
                        op0=mybir.AluOpType.add,
                    )
                    if qo == OGRP - 1 or j == nslots - 1:
                        ow = (q - qo) * QUAD * P
                        nc.gpsimd.dma_start(
                            out_d[:, ow:n0 + w],
                            o_sb[:, 0:n0 + w - ow],
                        )
    nc.finalize()
    return nc


def _schedule(col, n_nodes):
    """Assign 128-node blocks to (core, slot) and derive the shared
    per-slot tile schedule."""
    nblk = -(-n_nodes // P)
    nslots = -(-nblk // NCORES)
    nblk_pad = nslots * NCORES
    nsh = nslots * P

    blk = (col >> 7).astype(np.int64)
    order = np.argsort(blk, kind="stable")
    bc = np.bincount(blk, minlength=nblk_pad)
    bstart = np.zeros(nblk_pad + 1, np.int64)
    np.cumsum(bc, out=bstart[1:])

    sorted_blocks = np.argsort(-bc, kind="stable")
    blk_assign = sorted_blocks.reshape(nslots, NCORES)   # [slot, core]
    grp_max = bc[blk_assign].max(axis=1)
    t_sched = [int(v) for v in np.maximum(1, -(-grp_max // P))]
    return t_sched, blk_assign, order, bc, bstart, nslots, nsh


def kernel(x, edge_index, edge_attr, u, node_batch, wts,
           mw1, mb1, mw2, mb2, uw1, ub1, uw2, ub2):
    x = np.asarray(x, np.float32)
    edge_index = np.asarray(edge_index)
    edge_attr = np.asarray(edge_attr, np.float32)
    u = np.asarray(u, np.float32)
    node_batch = np.asarray(node_batch).astype(np.int64)
    wts = np.asarray(wts, np.float32).reshape(-1)
    mw1 = np.asarray(mw1, np.float32)
    mb1 = np.asarray(mb1, np.float32)
    mw2 = np.asarray(mw2, np.float32)
    mb2 = np.asarray(mb2, np.float32)
    uw1 = np.asarray(uw1, np.float32)
    ub1 = np.asarray(ub1, np.float32)
    uw2 = np.asarray(uw2, np.float32)
    ub2 = np.asarray(ub2, np.float32)

    n_nodes = x.shape[0]
    row = np.asarray(edge_index[0], np.int64)
    col = np.asarray(edge_index[1], np.int64)

    sched = _schedule(col, n_nodes)
    (t_sched, blk_assign, order, bc, bstart, nslots, nsh) = sched
    nt = sum(t_sched)
    e_pad = nt * P

    # per-node stats (host): count, 1/max(cnt,1), weight-sum
    cnt = np.bincount(col, minlength=n_nodes).astype(np.float32)
    rc = 1.0 / np.maximum(cnt, 1.0)
    wsum = np.bincount(col, weights=wts, minlength=n_nodes).astype(np.float32)
    s_node = wsum * rc

    # per-edge
    colof = (col & 127).astype(np.float32)
    wrc = wts * rc[col]

    key = tuple(t_sched)
    if key not in _program_cache:
        _program_cache[key] = _build_program(t_sched)
    nc = _program_cache[key]

    # const blob (shared by all cores)
    v_row = mb2 @ uw1[NODE_D:2 * NODE_D, :]              # [HID]
    wc = mw2 @ uw1[NODE_D:2 * NODE_D, :]                 # [HID, HID]
    blob = np.zeros((P, BLOB_W), np.float32)
    blob[0:NODE_D + EDGE_D, _B_MW1:_B_MW1 + HID] = mw1
    blob[NODE_D + EDGE_D, _B_MW1:_B_MW1 + HID] = mb1
    blob[0:NODE_D, _B_UW1:_B_UW1 + HID] = uw1[0:NODE_D, :]
    blob[NODE_D:NODE_D + GLOB_D, _B_UW1:_B_UW1 + HID] = uw1[2 * NODE_D:, :]
    blob[NODE_D + GLOB_D, _B_UW1:_B_UW1 + HID] = v_row
    blob[NODE_D + GLOB_D + 1, _B_UW1:_B_UW1 + HID] = ub1
    blob[0:HID, _B_WC:_B_WC + HID] = wc
    blob[0:HID, _B_UW2:_B_UW2 + 64] = uw2
    for nk in range(1, 5):
        c0 = _B_IR[nk - 1]
        blob[:, c0:c0 + nk * P] = (
            np.arange(nk * P, dtype=np.float32) // nk)[None, :]
    blob_bf = blob.astype(bf16)
    ub2_a = ub2.reshape(64, 1).astype(np.float32)

    u_per_node = u[node_batch]                           # [N, GLOB_D]

    slot_off = np.zeros(nslots + 1, np.int64)
    np.cumsum(np.asarray(t_sched) * P, out=slot_off[1:])

    in_maps = []
    node_idx_cores = []
    for c in range(NCORES):
        eidx = np.full(e_pad, -1, np.int64)
        nidx = np.full(nsh, -1, np.int64)
        for j in range(nslots):
            b = int(blk_assign[j, c])
            m = int(bc[b])
            o = slot_off[j]
            eidx[o:o + m] = order[bstart[b]:bstart[b] + m]
            n0 = b * P
            nn = min(P, n_nodes - n0)
            if nn > 0:
                nidx[j * P:j * P + nn] = np.arange(n0, n0 + nn)
        evalid = eidx >= 0
        eidxc = np.where(evalid, eidx, 0)
        # eax: [x[row] | edge_attr | 1] * wrc transposed, zeros on pads
        eax = np.empty((e_pad, FEAT), np.float32)
        eax[:, 0:NODE_D] = x[row[eidxc]]
        eax[:, NODE_D:NODE_D + EDGE_D] = edge_attr[eidxc]
        eax[:, NODE_D + EDGE_D] = 1.0
        eax[:, FEAT - 1] = 0.0
        wcol = np.where(evalid, wrc[eidxc], 0.0).astype(np.float32)
        eax *= wcol[:, None]
        cwv = np.full(e_pad, -1.0, np.float32)
        cwv[evalid] = colof[eidxc[evalid]]

        nvalid = nidx >= 0
        nidxc = np.where(nvalid, nidx, 0)
        xcon = np.zeros((nsh, XCON_R), np.float32)
        xcon[:, 0:NODE_D] = x[nidxc]
        xcon[:, NODE_D:NODE_D + GLOB_D] = u_per_node[nidxc]
        xcon[:, NODE_D + GLOB_D] = s_node[nidxc]
        xcon[:, NODE_D + GLOB_D + 1] = 1.0
        xcon[~nvalid] = 0.0

        in_maps.append({
            "eax": np.ascontiguousarray(eax.T).astype(bf16),
            "cwv": np.ascontiguousarray(
                cwv.reshape(nt, P).T).astype(bf16),
            "blob": blob_bf,
            "ub2": ub2_a,
            "xcon": np.ascontiguousarray(xcon.T).astype(bf16),
        })
        node_idx_cores.append((nidx, nvalid))

    res = run_bass_kernel_spmd(nc, in_maps, core_ids=list(range(NCORES)))
    global _last_results
    _last_results = res

    out_full = np.zeros((n_nodes, 64), np.float32)
    for c in range(NCORES):
        nidx, nvalid = node_idx_cores[c]
        oc = np.asarray(res.results[c]["out"], np.float32)   # [64, nsh]
        out_full[nidx[nvalid]] = oc.T[nvalid]
    return out_full
